# revision 20
# baseline (speedup 1.0000x reference)
"""Trainium2 Bass kernel for nn_DiverseRegDCConv2d.

Per-sample dynamic 3x3 conv: filters are generated per sample from an
8-column weight bank (wgen[b] = se[b] @ bank.T), then applied as a
standard 256->256 conv on 28x28 with padding 1.

Sharding (8 cores): 4 batch-groups x 2 out-channel halves. Each core
handles 8 samples x 128 out channels; the weight bank half it needs is
replicated across the 4 batch-groups. No cross-device communication.

Compute strategy (fp8 DoubleRow, 3-pass residual):
  The conv contraction (256 input channels = 2x128) runs as fp8
  DoubleRow matmuls, which process both 128-channel tiles in a single
  instruction at 2x the fp16 rate (4x overall).  Plain e4m3 is not
  accurate enough (rel err 3.7e-2 vs the 2e-2 gate), so operands are
  split hi/lo and three DoubleRow passes accumulate into one PSUM
  group:
      P1: Whi . xhi     (e4m3 x e4m3)  main term
      P2: Whi . xlo     (e4m3 x e4m3)  x-quantization correction
      P3: Wlo . xhi     (e5m2 x e4m3)  W-quantization correction
  Wlo must be e5m2: the residual magnitudes (~1e-3) underflow e4m3's
  denormal range.  Measured end-to-end rel err: ~1.6e-3.

  x windows stream as FLAT 420-wide slices of a row-contiguous
  [128, cc, 31, 30] image (padded, plus a zero guard row), offset by
  ky*30+kx per kernel position; the wrap-around junk lands in output
  columns 28/29 which the evacuation discards.  Flatness is required
  by the DoubleRow ifmap layout ([K, 2, N] with N flat).

  Filters are generated on-device in fp16 (bank tiles x block-diagonal
  se operand -> PSUM fp32, as in the fp16 kernel), then split into the
  e4m3 hi slab (activation copy) and e5m2 lo slab (tensor_tensor
  subtract), spread across Act/DVE/Pool so the split keeps pace with
  the PE during the generation phase.
"""

import sys

for _p in ("/opt/trn_rl_repo", "/root/.axon_site/_ro/trn_rl_repo"):
    if _p not in sys.path:
        sys.path.append(_p)

import numpy as np
import ml_dtypes

import concourse.bass as bass
import concourse.mybir as mybir
from concourse import bacc
from concourse.bass_utils import run_bass_kernel_spmd
from concourse.tile import TileContext

B, C, O, KS, H, W, NUM = 32, 256, 256, 3, 28, 28, 8
P = 128
NCORES = 8
BG, OHALF = 4, 2          # batch-groups x out-channel halves
S = B // BG               # samples per core = 8
OC = O // OHALF           # out channels per core = 128
CC = C // P               # input-channel chunks = 2
G = 16                    # (k,o)-blocks per wgen matmul (with NUM=8 fills K=128)
NP = KS * KS * OC         # (k, o_local) pairs per c-chunk = 1152
NM = NP // G              # wgen matmuls per c-chunk = 72
WR, WC = H + 3, W + 2     # stored x rows (pad + zero guard row) x cols = 31x30
HH = H // 2               # output rows per PSUM group = 14
NW = HH * WC              # flat moving-stream width = 420
F32 = mybir.dt.float32
F16 = mybir.dt.float16
E4 = mybir.dt.float8e4
E5 = mybir.dt.float8e5
IDENT = mybir.ActivationFunctionType.Identity
SUB = mybir.AluOpType.subtract

# -- schedule tuning knobs ------------------------------------------------
WAVE1 = 5          # conv groups interleaved with the wgen phase (PSUM tags)
ST12 = 2           # P1/P2 emitted this many k-blocks behind wgen
ST3 = 3            # P3 lag (waits on the longer hi->sub chain)

_NC = None


def _build_nc():
    nc = bacc.Bacc()
    x_d = nc.declare_dram_parameter("x", [S, 2, CC, P, WR * WC], E4, isOutput=False)
    wp_d = nc.declare_dram_parameter("wp", [P, CC * NM, P], F16, isOutput=False)
    se_d = nc.declare_dram_parameter("sebd", [P, P], F16, isOutput=False)
    b_d = nc.declare_dram_parameter("bias", [P, 1], F32, isOutput=False)
    out_d = nc.declare_dram_parameter("out", [S, P, H * W], F16, isOutput=True)

    with TileContext(nc) as tc:
        with (
            tc.tile_pool(name="constp", bufs=1) as constp,
            tc.tile_pool(name="wstream", bufs=18) as wstream,
            tc.tile_pool(name="slabp", bufs=1) as slabp,
            tc.tile_pool(name="xpool", bufs=1) as xpool,
            tc.tile_pool(name="outp", bufs=4) as outp,
            tc.tile_pool(name="psp", bufs=1, space="PSUM") as psp,
        ):
            se_sb = constp.tile([P, P], F16)
            nc.sync.dma_start(out=se_sb, in_=se_d[:, :])
            bias_sb = constp.tile([P, 1], F32)  # DMA deferred past startup

            # filter slabs: [c_part, cc, k, s, o]; conv lhsT slices are
            # wg*[:, :, k, s, :] = [128, 2, 128], the DoubleRow pair shape.
            wg_hi = slabp.tile([P, CC, KS * KS, S, P], E4)
            wg_lo = slabp.tile([P, CC, KS * KS, S, P], E5)

            # x tiles: [c_part, hi/lo, cc, 930] flat row-contiguous images
            xt = [
                xpool.tile([P, 2, CC, WR * WC], E4, name=f"xt_{s}", tag=f"xt_{s}")
                for s in range(S)
            ]
            xdone = set()

            def emit_xload(s):
                if s in xdone:
                    return
                xdone.add(s)
                nc.sync.dma_start(
                    out=xt[s], in_=x_d[s].rearrange("hl cc p w -> p hl cc w"),
                )

            def emit_wload(cc, k, split=False):
                t0 = cc * NM + k * 8
                wtb = wstream.tile([P, 8, P], F16, name=f"wtb_{cc}_{k}", tag="wtb")
                if split:
                    # halves land separately so the first wgen group starts
                    # as soon as tiles 0-3 arrive
                    nc.sync.dma_start(out=wtb[:, 0:4, :], in_=wp_d[:, t0:t0 + 4, :])
                    nc.sync.dma_start(out=wtb[:, 4:8, :], in_=wp_d[:, t0 + 4:t0 + 8, :])
                else:
                    nc.sync.dma_start(out=wtb, in_=wp_d[:, t0:t0 + 8, :])
                return wtb

            # wgen psum rotates over 3 of the shared pool's 8 bank tags;
            # phase-2 conv groups inherit those tags once wgen drains
            _splitn = [0]

            def emit_wgen(cc, k, wtb):
                # produce wg_hi/wg_lo[:, cc, k, :, :] (8 o_hi blocks = 2 groups)
                for j in range(2):
                    n = _splitn[0]
                    ps = psp.tile([P, 4 * P], F32, name=f"wgps_{n}",
                                  tag=f"wg{n % 3}")
                    for i in range(4):
                        nc.tensor.matmul(
                            ps[:, i * P:(i + 1) * P], wtb[:, j * 4 + i, :],
                            se_sb, start=True, stop=True,
                        )
                    # psum free layout: (o_hi, s, g); slab wants (s, o_hi, g)
                    oh0 = j * 4
                    src = ps.rearrange("p (oh s g) -> p oh s g", oh=4, s=S, g=G)
                    dhi = wg_hi[:, cc, k, :, oh0 * G:(oh0 + 4) * G].rearrange(
                        "p s (oh g) -> p oh s g", g=G)
                    dlo = wg_lo[:, cc, k, :, oh0 * G:(oh0 + 4) * G].rearrange(
                        "p s (oh g) -> p oh s g", g=G)
                    nc.scalar.activation(dhi, src, IDENT)
                    nc.vector.tensor_tensor(dlo, src, dhi, SUB)

            # conv psum tag order: 5 dedicated tags during phase 1, then the
            # freed wgen tags join the rotation (8 banks deep in phase 2)

            _cv_tags = [f"cv{i}" for i in range(WAVE1)] + ["wg0", "wg1", "wg2"]
            _tag = [0]

            def conv_psum():
                t = psp.tile([P, NW], F32, name=f"cps_{_tag[0]}",
                              tag=_cv_tags[_tag[0] % len(_cv_tags)])
                _tag[0] += 1
                return t

            def emit_conv_mm(k, s, hi, pst, pas, first, last):
                # pas 0: Whi.xhi  1: Whi.xlo  2: Wlo.xhi
                ky, kx = k // KS, k % KS
                off = (hi * HH + ky) * WC + kx
                hl = 1 if pas == 1 else 0
                wg = wg_lo if pas == 2 else wg_hi
                nc.tensor.matmul(
                    pst, wg[:, :, k, s, :], xt[s][:, hl, :, off:off + NW],
                    start=first, stop=last,
                    perf_mode=mybir.MatmulPerfMode.DoubleRow,
                    skip_group_check=True,
                )

            _evacn = [0]
            _ots = {}

            def emit_group_evac(s, hi, pst):
                if s not in _ots:
                    _ots[s] = outp.tile([P, 2, HH * W], F16, name=f"ot_{s}",
                                        tag=f"ot_{s % 4}")
                ot = _ots[s]
                src = pst.rearrange("p (r c) -> p r c", r=HH)[:, :, 0:W]
                dst = ot[:, hi, :].rearrange("p (r c) -> p r c", r=HH)
                if _evacn[0] % 2 == 0:
                    nc.scalar.activation(dst, src, IDENT, bias=bias_sb[:, 0:1])
                else:
                    nc.vector.tensor_scalar(
                        out=dst, in0=src, scalar1=bias_sb[:, 0:1],
                        scalar2=None, op0=mybir.AluOpType.add)
                _evacn[0] += 1
                if s == S - 1:
                    # last sample: per-half stores so the tail DMA is short
                    nc.sync.dma_start(
                        out=out_d[s, :, hi * HH * W:(hi + 1) * HH * W],
                        in_=ot[:, hi, :])
                elif hi == 1:
                    nc.sync.dma_start(out=out_d[s], in_=ot.rearrange("p h w -> p (h w)"))
                return ot

            # ---- phase 1: wgen + wave-1 conv groups, interleaved ----
            wave1 = [(s, hi) for s in range(WAVE1 // 2) for hi in range(2)][:WAVE1]
            prog = {}
            done_k = {}

            def open_group(g):
                prog[g] = conv_psum()
                done_k[g] = 0

            def conv_step(g, k, passes):
                # emit `passes` for kernel position k on group g
                s, hi = g
                pst = prog[g]
                for pas in passes:
                    idx = done_k[g]
                    emit_conv_mm(k, s, hi, pst, pas,
                                 first=(idx == 0), last=(idx == 27 - 1))
                    done_k[g] += 1

            for g in wave1:
                open_group(g)

            # DMA priority order: first wp pair, then early x, then the
            # rest of the weight stream with remaining x interleaved
            wtbs = {}
            wtbs[(0, 0)] = emit_wload(0, 0, split=True)
            wtbs[(1, 0)] = emit_wload(1, 0)
            emit_xload(0)
            nc.sync.dma_start(out=bias_sb, in_=b_d[:, :])
            emit_xload(1)
            for k in range(1, KS * KS):
                wtbs[(0, k)] = emit_wload(0, k)
                wtbs[(1, k)] = emit_wload(1, k)
                if k == 1:
                    emit_xload(2)
                elif k == 3:
                    emit_xload(3)
                elif k == 5:
                    emit_xload(4)
                elif k == 7:
                    emit_xload(5)
            emit_xload(6)
            emit_xload(7)

            for k in range(KS * KS):
                emit_wgen(0, k, wtbs[(0, k)])
                emit_wgen(1, k, wtbs[(1, k)])
                if k >= ST12:
                    for g in wave1:
                        conv_step(g, k - ST12, (0, 1))
                if k >= ST3:
                    for g in wave1:
                        conv_step(g, k - ST3, (2,))
            # drain wave-1 per group so PSUM tags free progressively
            for g in wave1:
                for kk in range(KS * KS - ST12, KS * KS):
                    conv_step(g, kk, (0, 1))
                for kk in range(KS * KS - ST3, KS * KS):
                    conv_step(g, kk, (2,))
                emit_group_evac(*g, prog[g])

            # ---- phase 2: remaining groups, straight bursts ----
            rest = [(s, hi) for s in range(S) for hi in range(2)
                    if (s, hi) not in set(wave1)]
            for g in rest:
                open_group(g)
                for k in range(KS * KS):
                    conv_step(g, k, (0, 1, 2))
                emit_group_evac(*g, prog[g])

    nc.compile()
    return nc


def _get_nc():
    global _NC
    if _NC is None:
        _NC = _build_nc()
    return _NC


def _prep_core_inputs(inputs, inputs_se, weight, bias, bg, oh):
    # weight rows: r = o*(C*9) + c*9 + (ky*3+kx)  -> [O, C, 3, 3, NUM]
    wr = weight.reshape(O, C, KS, KS, NUM)
    wo = wr[oh * OC:(oh + 1) * OC]            # [128, 256, 3, 3, 8]
    p_arr = np.arange(NP)
    k_arr = p_arr // OC                       # k index per (m,g) pair
    o_arr = p_arr % OC
    t = wo[o_arr, :, k_arr // KS, k_arr % KS, :]     # [1152, 256, 8]
    wp = (
        t.reshape(NM, G, CC, P, NUM)
        .transpose(2, 0, 4, 1, 3)             # cc, m, n, g, c
        .reshape(CC * NM, P, P)
        .transpose(1, 0, 2)                   # p-major for contiguous DMA
    )
    wp = np.ascontiguousarray(wp.astype(np.float16))

    se_core = inputs_se[bg * S:(bg + 1) * S]  # [8, 8] (s, n)
    sebd = np.zeros((NUM, G, S, G), dtype=np.float32)
    for g in range(G):
        sebd[:, g, :, g] = se_core.T
    sebd = sebd.reshape(P, P).astype(np.float16)

    # padded x + zero guard row, flat rows; hi/lo e4m3 split
    xp = np.pad(
        inputs[bg * S:(bg + 1) * S], ((0, 0), (0, 0), (1, 2), (1, 1))
    ).reshape(S, CC, P, WR * WC)
    xhi = xp.astype(ml_dtypes.float8_e4m3)
    xlo = (xp - xhi.astype(np.float32)).astype(ml_dtypes.float8_e4m3)
    x_arr = np.stack([xhi, xlo], axis=1)      # [S, 2, CC, P, 930]

    return {
        "x": np.ascontiguousarray(x_arr),
        "wp": wp,
        "sebd": sebd,
        "bias": np.ascontiguousarray(
            bias[oh * OC:(oh + 1) * OC].reshape(OC, 1), dtype=np.float32
        ),
    }


def kernel(inputs, inputs_se, weight, bias):
    inputs = np.asarray(inputs, dtype=np.float32)
    inputs_se = np.asarray(inputs_se, dtype=np.float32)
    weight = np.asarray(weight, dtype=np.float32)
    bias = np.asarray(bias, dtype=np.float32)

    nc = _get_nc()
    in_maps = []
    for core in range(NCORES):
        bg, oh = core // OHALF, core % OHALF
        in_maps.append(_prep_core_inputs(inputs, inputs_se, weight, bias, bg, oh))

    res = run_bass_kernel_spmd(nc, in_maps, list(range(NCORES))).results

    out = np.empty((B, O, H, W), dtype=np.float32)
    for core in range(NCORES):
        bg, oh = core // OHALF, core % OHALF
        out[bg * S:(bg + 1) * S, oh * OC:(oh + 1) * OC] = (
            res[core]["out"].astype(np.float32).reshape(S, OC, H, W)
        )
    return out


# revision 21
# speedup vs baseline: 1.6367x; 1.6367x over previous
"""Trainium2 Bass kernel for nn_DiverseRegDCConv2d.

Per-sample dynamic 3x3 conv: filters are generated per sample from an
8-column weight bank (wgen[b] = se[b] @ bank.T), then applied as a
standard 256->256 conv on 28x28 with padding 1.

Sharding (8 cores): 4 batch-groups x 2 out-channel halves. Each core
handles 8 samples x 128 out channels; the weight bank half it needs is
replicated across the 4 batch-groups. No cross-device communication.

Compute strategy (fp8 DoubleRow, 3-pass residual):
  The conv contraction (256 input channels = 2x128) runs as fp8
  DoubleRow matmuls, which process both 128-channel tiles in a single
  instruction at 2x the fp16 rate (4x overall).  Plain e4m3 is not
  accurate enough (rel err 3.7e-2 vs the 2e-2 gate), so operands are
  split hi/lo and three DoubleRow passes accumulate into one PSUM
  group:
      P1: Whi . xhi     (e4m3 x e4m3)  main term
      P2: Whi . xlo     (e4m3 x e4m3)  x-quantization correction
      P3: Wlo . xhi     (e5m2 x e4m3)  W-quantization correction
  Wlo must be e5m2: the residual magnitudes (~1e-3) underflow e4m3's
  denormal range.  Measured end-to-end rel err: ~1.6e-3.

  x windows stream as FLAT 420-wide slices of a row-contiguous
  [128, cc, 31, 30] image (padded, plus a zero guard row), offset by
  ky*30+kx per kernel position; the wrap-around junk lands in output
  columns 28/29 which the evacuation discards.  Flatness is required
  by the DoubleRow ifmap layout ([K, 2, N] with N flat).

  Filters are generated on-device in fp16 (bank tiles x block-diagonal
  se operand -> PSUM fp32, as in the fp16 kernel), then split into the
  e4m3 hi slab (activation copy) and e5m2 lo slab (tensor_tensor
  subtract), spread across Act/DVE/Pool so the split keeps pace with
  the PE during the generation phase.
"""

import sys

for _p in ("/opt/trn_rl_repo", "/root/.axon_site/_ro/trn_rl_repo"):
    if _p not in sys.path:
        sys.path.append(_p)

import numpy as np
import ml_dtypes

import concourse.bass as bass
import concourse.mybir as mybir
from concourse import bacc
from concourse.bass_utils import run_bass_kernel_spmd
from concourse.tile import TileContext

B, C, O, KS, H, W, NUM = 32, 256, 256, 3, 28, 28, 8
P = 128
NCORES = 8
BG, OHALF = 4, 2          # batch-groups x out-channel halves
S = B // BG               # samples per core = 8
OC = O // OHALF           # out channels per core = 128
CC = C // P               # input-channel chunks = 2
G = 16                    # (k,o)-blocks per wgen matmul (with NUM=8 fills K=128)
NP = KS * KS * OC         # (k, o_local) pairs per c-chunk = 1152
NM = NP // G              # wgen matmuls per c-chunk = 72
WR, WC = H + 3, W + 2     # stored x rows (pad + zero guard row) x cols = 31x30
HH = H // 2               # output rows per PSUM group = 14
NW = HH * WC              # flat moving-stream width = 420
F32 = mybir.dt.float32
F16 = mybir.dt.float16
E4 = mybir.dt.float8e4
E5 = mybir.dt.float8e5
IDENT = mybir.ActivationFunctionType.Identity
SUB = mybir.AluOpType.subtract

# -- schedule tuning knobs ------------------------------------------------
WAVE1 = 5          # conv groups interleaved with the wgen phase (PSUM tags)
ST12 = 2           # P1/P2 emitted this many k-blocks behind wgen
ST3 = 3            # P3 lag (waits on the longer hi->sub chain)

_NC = None


def _build_nc():
    nc = bacc.Bacc()
    x_d = nc.declare_dram_parameter("x", [S, 2, CC, P, WR * WC], E4, isOutput=False)
    wp_d = nc.declare_dram_parameter("wp", [P, CC * NM, P], F16, isOutput=False)
    se_d = nc.declare_dram_parameter("sebd", [P, P], F16, isOutput=False)
    b_d = nc.declare_dram_parameter("bias", [P, 1], F32, isOutput=False)
    out_d = nc.declare_dram_parameter("out", [S, P, H * W], F16, isOutput=True)

    with TileContext(nc) as tc:
        with (
            tc.tile_pool(name="constp", bufs=1) as constp,
            tc.tile_pool(name="wstream", bufs=18) as wstream,
            tc.tile_pool(name="slabp", bufs=1) as slabp,
            tc.tile_pool(name="xpool", bufs=1) as xpool,
            tc.tile_pool(name="outp", bufs=4) as outp,
            tc.tile_pool(name="psp", bufs=1, space="PSUM") as psp,
        ):
            se_sb = constp.tile([P, P], F16)
            nc.sync.dma_start(out=se_sb, in_=se_d[:, :])
            bias_sb = constp.tile([P, 1], F32)  # DMA deferred past startup

            # filter slabs: [c_part, cc, k, s, o]; conv lhsT slices are
            # wg*[:, :, k, s, :] = [128, 2, 128], the DoubleRow pair shape.
            wg_hi = slabp.tile([P, CC, KS * KS, S, P], E4)
            wg_lo = slabp.tile([P, CC, KS * KS, S, P], E5)

            # x tiles: [c_part, hi/lo, cc, 930] flat row-contiguous images
            xt = [
                xpool.tile([P, 2, CC, WR * WC], E4, name=f"xt_{s}", tag=f"xt_{s}")
                for s in range(S)
            ]
            xdone = set()

            def emit_xload(s):
                if s in xdone:
                    return
                xdone.add(s)
                nc.sync.dma_start(
                    out=xt[s], in_=x_d[s].rearrange("hl cc p w -> p hl cc w"),
                )

            def emit_wload(cc, k, split=False):
                t0 = cc * NM + k * 8
                wtb = wstream.tile([P, 8, P], F16, name=f"wtb_{cc}_{k}", tag="wtb")
                if split:
                    # halves land separately so the first wgen group starts
                    # as soon as tiles 0-3 arrive
                    nc.sync.dma_start(out=wtb[:, 0:4, :], in_=wp_d[:, t0:t0 + 4, :])
                    nc.sync.dma_start(out=wtb[:, 4:8, :], in_=wp_d[:, t0 + 4:t0 + 8, :])
                else:
                    nc.sync.dma_start(out=wtb, in_=wp_d[:, t0:t0 + 8, :])
                return wtb

            # wgen psum rotates over 3 of the shared pool's 8 bank tags;
            # phase-2 conv groups inherit those tags once wgen drains
            _splitn = [0]

            def emit_wgen(cc, k, wtb):
                # produce wg_hi/wg_lo[:, cc, k, :, :] (8 o_hi blocks = 2 groups)
                for j in range(2):
                    n = _splitn[0]
                    _splitn[0] += 1
                    ps = psp.tile([P, 4 * P], F32, name=f"wgps_{n}",
                                  tag=f"wg{n % 3}")
                    for i in range(4):
                        nc.tensor.matmul(
                            ps[:, i * P:(i + 1) * P], wtb[:, j * 4 + i, :],
                            se_sb, start=True, stop=True,
                        )
                    # psum free layout: (o_hi, s, g); slab wants (s, o_hi, g)
                    oh0 = j * 4
                    src = ps.rearrange("p (oh s g) -> p oh s g", oh=4, s=S, g=G)
                    dhi = wg_hi[:, cc, k, :, oh0 * G:(oh0 + 4) * G].rearrange(
                        "p s (oh g) -> p oh s g", g=G)
                    dlo = wg_lo[:, cc, k, :, oh0 * G:(oh0 + 4) * G].rearrange(
                        "p s (oh g) -> p oh s g", g=G)
                    nc.scalar.activation(dhi, src, IDENT)
                    nc.vector.tensor_tensor(dlo, src, dhi, SUB)

            # conv psum tag order: 5 dedicated tags during phase 1, then the
            # freed wgen tags join the rotation (8 banks deep in phase 2)

            _cv_tags = [f"cv{i}" for i in range(WAVE1)] + ["wg0", "wg1", "wg2"]
            _tag = [0]

            def conv_psum():
                t = psp.tile([P, NW], F32, name=f"cps_{_tag[0]}",
                              tag=_cv_tags[_tag[0] % len(_cv_tags)])
                _tag[0] += 1
                return t

            def emit_conv_mm(k, s, hi, pst, pas, first, last):
                # pas 0: Whi.xhi  1: Whi.xlo  2: Wlo.xhi
                ky, kx = k // KS, k % KS
                off = (hi * HH + ky) * WC + kx
                hl = 1 if pas == 1 else 0
                wg = wg_lo if pas == 2 else wg_hi
                nc.tensor.matmul(
                    pst, wg[:, :, k, s, :], xt[s][:, hl, :, off:off + NW],
                    start=first, stop=last,
                    perf_mode=mybir.MatmulPerfMode.DoubleRow,
                    skip_group_check=True,
                )

            _evacn = [0]
            _ots = {}

            def emit_group_evac(s, hi, pst):
                if s not in _ots:
                    _ots[s] = outp.tile([P, 2, HH * W], F16, name=f"ot_{s}",
                                        tag=f"ot_{s % 4}")
                ot = _ots[s]
                src = pst.rearrange("p (r c) -> p r c", r=HH)[:, :, 0:W]
                dst = ot[:, hi, :].rearrange("p (r c) -> p r c", r=HH)
                if _evacn[0] % 2 == 0:
                    nc.scalar.activation(dst, src, IDENT, bias=bias_sb[:, 0:1])
                else:
                    nc.vector.tensor_scalar(
                        out=dst, in0=src, scalar1=bias_sb[:, 0:1],
                        scalar2=None, op0=mybir.AluOpType.add)
                _evacn[0] += 1
                if s == S - 1:
                    # last sample: per-half stores so the tail DMA is short
                    nc.sync.dma_start(
                        out=out_d[s, :, hi * HH * W:(hi + 1) * HH * W],
                        in_=ot[:, hi, :])
                elif hi == 1:
                    nc.sync.dma_start(out=out_d[s], in_=ot.rearrange("p h w -> p (h w)"))
                return ot

            # ---- phase 1: wgen + wave-1 conv groups, interleaved ----
            wave1 = [(s, hi) for s in range(WAVE1 // 2) for hi in range(2)][:WAVE1]
            prog = {}
            done_k = {}

            def open_group(g):
                prog[g] = conv_psum()
                done_k[g] = 0

            def conv_step(g, k, passes):
                # emit `passes` for kernel position k on group g
                s, hi = g
                pst = prog[g]
                for pas in passes:
                    idx = done_k[g]
                    emit_conv_mm(k, s, hi, pst, pas,
                                 first=(idx == 0), last=(idx == 27 - 1))
                    done_k[g] += 1

            for g in wave1:
                open_group(g)

            # DMA priority order: first wp pair, then early x, then the
            # rest of the weight stream with remaining x interleaved
            wtbs = {}
            wtbs[(0, 0)] = emit_wload(0, 0, split=True)
            wtbs[(1, 0)] = emit_wload(1, 0)
            emit_xload(0)
            nc.sync.dma_start(out=bias_sb, in_=b_d[:, :])
            emit_xload(1)
            for k in range(1, KS * KS):
                wtbs[(0, k)] = emit_wload(0, k)
                wtbs[(1, k)] = emit_wload(1, k)
                if k == 1:
                    emit_xload(2)
                elif k == 3:
                    emit_xload(3)
                elif k == 5:
                    emit_xload(4)
                elif k == 7:
                    emit_xload(5)
            emit_xload(6)
            emit_xload(7)

            for k in range(KS * KS):
                emit_wgen(0, k, wtbs[(0, k)])
                emit_wgen(1, k, wtbs[(1, k)])
                if k >= ST12:
                    for g in wave1:
                        conv_step(g, k - ST12, (0, 1))
                if k >= ST3:
                    for g in wave1:
                        conv_step(g, k - ST3, (2,))
            # drain wave-1 per group so PSUM tags free progressively
            for g in wave1:
                for kk in range(KS * KS - ST12, KS * KS):
                    conv_step(g, kk, (0, 1))
                for kk in range(KS * KS - ST3, KS * KS):
                    conv_step(g, kk, (2,))
                emit_group_evac(*g, prog[g])

            # ---- phase 2: remaining groups, straight bursts ----
            rest = [(s, hi) for s in range(S) for hi in range(2)
                    if (s, hi) not in set(wave1)]
            for g in rest:
                open_group(g)
                for k in range(KS * KS):
                    conv_step(g, k, (0, 1, 2))
                emit_group_evac(*g, prog[g])

    nc.compile()
    return nc


def _get_nc():
    global _NC
    if _NC is None:
        _NC = _build_nc()
    return _NC


def _prep_core_inputs(inputs, inputs_se, weight, bias, bg, oh):
    # weight rows: r = o*(C*9) + c*9 + (ky*3+kx)  -> [O, C, 3, 3, NUM]
    wr = weight.reshape(O, C, KS, KS, NUM)
    wo = wr[oh * OC:(oh + 1) * OC]            # [128, 256, 3, 3, 8]
    p_arr = np.arange(NP)
    k_arr = p_arr // OC                       # k index per (m,g) pair
    o_arr = p_arr % OC
    t = wo[o_arr, :, k_arr // KS, k_arr % KS, :]     # [1152, 256, 8]
    wp = (
        t.reshape(NM, G, CC, P, NUM)
        .transpose(2, 0, 4, 1, 3)             # cc, m, n, g, c
        .reshape(CC * NM, P, P)
        .transpose(1, 0, 2)                   # p-major for contiguous DMA
    )
    wp = np.ascontiguousarray(wp.astype(np.float16))

    se_core = inputs_se[bg * S:(bg + 1) * S]  # [8, 8] (s, n)
    sebd = np.zeros((NUM, G, S, G), dtype=np.float32)
    for g in range(G):
        sebd[:, g, :, g] = se_core.T
    sebd = sebd.reshape(P, P).astype(np.float16)

    # padded x + zero guard row, flat rows; hi/lo e4m3 split
    xp = np.pad(
        inputs[bg * S:(bg + 1) * S], ((0, 0), (0, 0), (1, 2), (1, 1))
    ).reshape(S, CC, P, WR * WC)
    xhi = xp.astype(ml_dtypes.float8_e4m3)
    xlo = (xp - xhi.astype(np.float32)).astype(ml_dtypes.float8_e4m3)
    x_arr = np.stack([xhi, xlo], axis=1)      # [S, 2, CC, P, 930]

    return {
        "x": np.ascontiguousarray(x_arr),
        "wp": wp,
        "sebd": sebd,
        "bias": np.ascontiguousarray(
            bias[oh * OC:(oh + 1) * OC].reshape(OC, 1), dtype=np.float32
        ),
    }


def kernel(inputs, inputs_se, weight, bias):
    inputs = np.asarray(inputs, dtype=np.float32)
    inputs_se = np.asarray(inputs_se, dtype=np.float32)
    weight = np.asarray(weight, dtype=np.float32)
    bias = np.asarray(bias, dtype=np.float32)

    nc = _get_nc()
    in_maps = []
    for core in range(NCORES):
        bg, oh = core // OHALF, core % OHALF
        in_maps.append(_prep_core_inputs(inputs, inputs_se, weight, bias, bg, oh))

    res = run_bass_kernel_spmd(nc, in_maps, list(range(NCORES))).results

    out = np.empty((B, O, H, W), dtype=np.float32)
    for core in range(NCORES):
        bg, oh = core // OHALF, core % OHALF
        out[bg * S:(bg + 1) * S, oh * OC:(oh + 1) * OC] = (
            res[core]["out"].astype(np.float32).reshape(S, OC, H, W)
        )
    return out


# revision 22
# speedup vs baseline: 1.6462x; 1.0058x over previous
"""Trainium2 Bass kernel for nn_DiverseRegDCConv2d.

Per-sample dynamic 3x3 conv: filters are generated per sample from an
8-column weight bank (wgen[b] = se[b] @ bank.T), then applied as a
standard 256->256 conv on 28x28 with padding 1.

Sharding (8 cores): 4 batch-groups x 2 out-channel halves. Each core
handles 8 samples x 128 out channels; the weight bank half it needs is
replicated across the 4 batch-groups. No cross-device communication.

Compute strategy (fp8 DoubleRow, 3-pass residual):
  The conv contraction (256 input channels = 2x128) runs as fp8
  DoubleRow matmuls, which process both 128-channel tiles in a single
  instruction at 2x the fp16 rate (4x overall).  Plain e4m3 is not
  accurate enough (rel err 3.7e-2 vs the 2e-2 gate), so operands are
  split hi/lo and three DoubleRow passes accumulate into one PSUM
  group:
      P1: Whi . xhi     (e4m3 x e4m3)  main term
      P2: Whi . xlo     (e4m3 x e4m3)  x-quantization correction
      P3: Wlo . xhi     (e5m2 x e4m3)  W-quantization correction
  Wlo must be e5m2: the residual magnitudes (~1e-3) underflow e4m3's
  denormal range.  Measured end-to-end rel err: ~1.6e-3.

  x windows stream as FLAT 420-wide slices of a row-contiguous
  [128, cc, 31, 30] image (padded, plus a zero guard row), offset by
  ky*30+kx per kernel position; the wrap-around junk lands in output
  columns 28/29 which the evacuation discards.  Flatness is required
  by the DoubleRow ifmap layout ([K, 2, N] with N flat).

  Filters are generated on-device in fp16 (bank tiles x block-diagonal
  se operand -> PSUM fp32, as in the fp16 kernel), then split into the
  e4m3 hi slab (activation copy) and e5m2 lo slab (tensor_tensor
  subtract), spread across Act/DVE/Pool so the split keeps pace with
  the PE during the generation phase.
"""

import sys

for _p in ("/opt/trn_rl_repo", "/root/.axon_site/_ro/trn_rl_repo"):
    if _p not in sys.path:
        sys.path.append(_p)

import numpy as np
import ml_dtypes

import concourse.bass as bass
import concourse.mybir as mybir
from concourse import bacc
from concourse.bass_utils import run_bass_kernel_spmd
from concourse.tile import TileContext

B, C, O, KS, H, W, NUM = 32, 256, 256, 3, 28, 28, 8
P = 128
NCORES = 8
BG, OHALF = 4, 2          # batch-groups x out-channel halves
S = B // BG               # samples per core = 8
OC = O // OHALF           # out channels per core = 128
CC = C // P               # input-channel chunks = 2
G = 16                    # (k,o)-blocks per wgen matmul (with NUM=8 fills K=128)
NP = KS * KS * OC         # (k, o_local) pairs per c-chunk = 1152
NM = NP // G              # wgen matmuls per c-chunk = 72
WR, WC = H + 3, W + 2     # stored x rows (pad + zero guard row) x cols = 31x30
HH = H // 2               # output rows per PSUM group = 14
NW = HH * WC              # flat moving-stream width = 420
F32 = mybir.dt.float32
F16 = mybir.dt.float16
E4 = mybir.dt.float8e4
E5 = mybir.dt.float8e5
IDENT = mybir.ActivationFunctionType.Identity
SUB = mybir.AluOpType.subtract

# -- schedule tuning knobs ------------------------------------------------
WAVE1 = 5          # conv groups interleaved with the wgen phase (PSUM tags)
ST12 = 1           # P1/P2 emitted this many k-blocks behind wgen
ST3 = 2            # P3 lag (waits on the longer hi->sub chain)

_NC = None


def _build_nc():
    nc = bacc.Bacc()
    x_d = nc.declare_dram_parameter("x", [S, 2, CC, P, WR * WC], E4, isOutput=False)
    wp_d = nc.declare_dram_parameter("wp", [P, CC * NM, P], F16, isOutput=False)
    se_d = nc.declare_dram_parameter("sebd", [P, P], F16, isOutput=False)
    b_d = nc.declare_dram_parameter("bias", [P, 1], F32, isOutput=False)
    out_d = nc.declare_dram_parameter("out", [S, P, H * W], F16, isOutput=True)

    with TileContext(nc) as tc:
        with (
            tc.tile_pool(name="constp", bufs=1) as constp,
            tc.tile_pool(name="wstream", bufs=18) as wstream,
            tc.tile_pool(name="slabp", bufs=1) as slabp,
            tc.tile_pool(name="xpool", bufs=1) as xpool,
            tc.tile_pool(name="outp", bufs=4) as outp,
            tc.tile_pool(name="psp", bufs=1, space="PSUM") as psp,
        ):
            se_sb = constp.tile([P, P], F16)
            nc.sync.dma_start(out=se_sb, in_=se_d[:, :])
            bias_sb = constp.tile([P, 1], F32)  # DMA deferred past startup

            # filter slabs: [c_part, cc, k, s, o]; conv lhsT slices are
            # wg*[:, :, k, s, :] = [128, 2, 128], the DoubleRow pair shape.
            wg_hi = slabp.tile([P, CC, KS * KS, S, P], E4)
            wg_lo = slabp.tile([P, CC, KS * KS, S, P], E5)

            # x tiles: [c_part, hi/lo, cc, 930] flat row-contiguous images
            xt = [
                xpool.tile([P, 2, CC, WR * WC], E4, name=f"xt_{s}", tag=f"xt_{s}")
                for s in range(S)
            ]
            xdone = set()

            def emit_xload(s):
                if s in xdone:
                    return
                xdone.add(s)
                nc.sync.dma_start(
                    out=xt[s], in_=x_d[s].rearrange("hl cc p w -> p hl cc w"),
                )

            def emit_wload(cc, k, split=False):
                t0 = cc * NM + k * 8
                wtb = wstream.tile([P, 8, P], F16, name=f"wtb_{cc}_{k}", tag="wtb")
                if split:
                    # halves land separately so the first wgen group starts
                    # as soon as tiles 0-3 arrive
                    nc.sync.dma_start(out=wtb[:, 0:4, :], in_=wp_d[:, t0:t0 + 4, :])
                    nc.sync.dma_start(out=wtb[:, 4:8, :], in_=wp_d[:, t0 + 4:t0 + 8, :])
                else:
                    nc.sync.dma_start(out=wtb, in_=wp_d[:, t0:t0 + 8, :])
                return wtb

            # wgen psum rotates over 3 of the shared pool's 8 bank tags;
            # phase-2 conv groups inherit those tags once wgen drains
            _splitn = [0]

            def emit_wgen(cc, k, wtb):
                # produce wg_hi/wg_lo[:, cc, k, :, :] (8 o_hi blocks = 2 groups)
                for j in range(2):
                    n = _splitn[0]
                    _splitn[0] += 1
                    ps = psp.tile([P, 4 * P], F32, name=f"wgps_{n}",
                                  tag=f"wg{n % 3}")
                    for i in range(4):
                        nc.tensor.matmul(
                            ps[:, i * P:(i + 1) * P], wtb[:, j * 4 + i, :],
                            se_sb, start=True, stop=True,
                        )
                    # psum free layout: (o_hi, s, g); slab wants (s, o_hi, g)
                    oh0 = j * 4
                    src = ps.rearrange("p (oh s g) -> p oh s g", oh=4, s=S, g=G)
                    dhi = wg_hi[:, cc, k, :, oh0 * G:(oh0 + 4) * G].rearrange(
                        "p s (oh g) -> p oh s g", g=G)
                    dlo = wg_lo[:, cc, k, :, oh0 * G:(oh0 + 4) * G].rearrange(
                        "p s (oh g) -> p oh s g", g=G)
                    nc.scalar.activation(dhi, src, IDENT)
                    nc.vector.tensor_tensor(dlo, src, dhi, SUB)

            # conv psum tag order: 5 dedicated tags during phase 1, then the
            # freed wgen tags join the rotation (8 banks deep in phase 2)

            _cv_tags = [f"cv{i}" for i in range(WAVE1)] + ["wg0", "wg1", "wg2"]
            _tag = [0]

            def conv_psum():
                t = psp.tile([P, NW], F32, name=f"cps_{_tag[0]}",
                              tag=_cv_tags[_tag[0] % len(_cv_tags)])
                _tag[0] += 1
                return t

            def emit_conv_mm(k, s, hi, pst, pas, first, last):
                # pas 0: Whi.xhi  1: Whi.xlo  2: Wlo.xhi
                ky, kx = k // KS, k % KS
                off = (hi * HH + ky) * WC + kx
                hl = 1 if pas == 1 else 0
                wg = wg_lo if pas == 2 else wg_hi
                nc.tensor.matmul(
                    pst, wg[:, :, k, s, :], xt[s][:, hl, :, off:off + NW],
                    start=first, stop=last,
                    perf_mode=mybir.MatmulPerfMode.DoubleRow,
                    skip_group_check=True,
                )

            _evacn = [0]
            _ots = {}

            def emit_group_evac(s, hi, pst):
                if s not in _ots:
                    _ots[s] = outp.tile([P, 2, HH * W], F16, name=f"ot_{s}",
                                        tag=f"ot_{s % 4}")
                ot = _ots[s]
                src = pst.rearrange("p (r c) -> p r c", r=HH)[:, :, 0:W]
                dst = ot[:, hi, :].rearrange("p (r c) -> p r c", r=HH)
                if _evacn[0] % 2 == 0:
                    nc.scalar.activation(dst, src, IDENT, bias=bias_sb[:, 0:1])
                else:
                    nc.vector.tensor_scalar(
                        out=dst, in0=src, scalar1=bias_sb[:, 0:1],
                        scalar2=None, op0=mybir.AluOpType.add)
                _evacn[0] += 1
                if s == S - 1:
                    # last sample: per-half stores so the tail DMA is short
                    nc.sync.dma_start(
                        out=out_d[s, :, hi * HH * W:(hi + 1) * HH * W],
                        in_=ot[:, hi, :])
                elif hi == 1:
                    nc.sync.dma_start(out=out_d[s], in_=ot.rearrange("p h w -> p (h w)"))
                return ot

            # ---- phase 1: wgen + wave-1 conv groups, interleaved ----
            wave1 = [(s, hi) for s in range(WAVE1 // 2) for hi in range(2)][:WAVE1]
            prog = {}
            done_k = {}

            def open_group(g):
                prog[g] = conv_psum()
                done_k[g] = 0

            def conv_step(g, k, passes):
                # emit `passes` for kernel position k on group g
                s, hi = g
                pst = prog[g]
                for pas in passes:
                    idx = done_k[g]
                    emit_conv_mm(k, s, hi, pst, pas,
                                 first=(idx == 0), last=(idx == 27 - 1))
                    done_k[g] += 1

            for g in wave1:
                open_group(g)

            # DMA priority order: first wp pair, then early x, then the
            # rest of the weight stream with remaining x interleaved
            wtbs = {}
            wtbs[(0, 0)] = emit_wload(0, 0, split=True)
            wtbs[(1, 0)] = emit_wload(1, 0)
            emit_xload(0)
            nc.sync.dma_start(out=bias_sb, in_=b_d[:, :])
            emit_xload(1)
            for k in range(1, KS * KS):
                wtbs[(0, k)] = emit_wload(0, k)
                wtbs[(1, k)] = emit_wload(1, k)
                if k == 1:
                    emit_xload(2)
                elif k == 3:
                    emit_xload(3)
                elif k == 5:
                    emit_xload(4)
                elif k == 7:
                    emit_xload(5)
            emit_xload(6)
            emit_xload(7)

            for k in range(KS * KS):
                emit_wgen(0, k, wtbs[(0, k)])
                emit_wgen(1, k, wtbs[(1, k)])
                if k >= ST12:
                    for g in wave1:
                        conv_step(g, k - ST12, (0, 1))
                if k >= ST3:
                    for g in wave1:
                        conv_step(g, k - ST3, (2,))
            # drain wave-1 per group so PSUM tags free progressively
            for g in wave1:
                for kk in range(KS * KS - ST12, KS * KS):
                    conv_step(g, kk, (0, 1))
                for kk in range(KS * KS - ST3, KS * KS):
                    conv_step(g, kk, (2,))
                emit_group_evac(*g, prog[g])

            # ---- phase 2: remaining groups, straight bursts ----
            rest = [(s, hi) for s in range(S) for hi in range(2)
                    if (s, hi) not in set(wave1)]
            for g in rest:
                open_group(g)
                for k in range(KS * KS):
                    conv_step(g, k, (0, 1, 2))
                emit_group_evac(*g, prog[g])

    nc.compile()
    return nc


def _get_nc():
    global _NC
    if _NC is None:
        _NC = _build_nc()
    return _NC


def _prep_core_inputs(inputs, inputs_se, weight, bias, bg, oh):
    # weight rows: r = o*(C*9) + c*9 + (ky*3+kx)  -> [O, C, 3, 3, NUM]
    wr = weight.reshape(O, C, KS, KS, NUM)
    wo = wr[oh * OC:(oh + 1) * OC]            # [128, 256, 3, 3, 8]
    p_arr = np.arange(NP)
    k_arr = p_arr // OC                       # k index per (m,g) pair
    o_arr = p_arr % OC
    t = wo[o_arr, :, k_arr // KS, k_arr % KS, :]     # [1152, 256, 8]
    wp = (
        t.reshape(NM, G, CC, P, NUM)
        .transpose(2, 0, 4, 1, 3)             # cc, m, n, g, c
        .reshape(CC * NM, P, P)
        .transpose(1, 0, 2)                   # p-major for contiguous DMA
    )
    wp = np.ascontiguousarray(wp.astype(np.float16))

    se_core = inputs_se[bg * S:(bg + 1) * S]  # [8, 8] (s, n)
    sebd = np.zeros((NUM, G, S, G), dtype=np.float32)
    for g in range(G):
        sebd[:, g, :, g] = se_core.T
    sebd = sebd.reshape(P, P).astype(np.float16)

    # padded x + zero guard row, flat rows; hi/lo e4m3 split
    xp = np.pad(
        inputs[bg * S:(bg + 1) * S], ((0, 0), (0, 0), (1, 2), (1, 1))
    ).reshape(S, CC, P, WR * WC)
    xhi = xp.astype(ml_dtypes.float8_e4m3)
    xlo = (xp - xhi.astype(np.float32)).astype(ml_dtypes.float8_e4m3)
    x_arr = np.stack([xhi, xlo], axis=1)      # [S, 2, CC, P, 930]

    return {
        "x": np.ascontiguousarray(x_arr),
        "wp": wp,
        "sebd": sebd,
        "bias": np.ascontiguousarray(
            bias[oh * OC:(oh + 1) * OC].reshape(OC, 1), dtype=np.float32
        ),
    }


def kernel(inputs, inputs_se, weight, bias):
    inputs = np.asarray(inputs, dtype=np.float32)
    inputs_se = np.asarray(inputs_se, dtype=np.float32)
    weight = np.asarray(weight, dtype=np.float32)
    bias = np.asarray(bias, dtype=np.float32)

    nc = _get_nc()
    in_maps = []
    for core in range(NCORES):
        bg, oh = core // OHALF, core % OHALF
        in_maps.append(_prep_core_inputs(inputs, inputs_se, weight, bias, bg, oh))

    res = run_bass_kernel_spmd(nc, in_maps, list(range(NCORES))).results

    out = np.empty((B, O, H, W), dtype=np.float32)
    for core in range(NCORES):
        bg, oh = core // OHALF, core % OHALF
        out[bg * S:(bg + 1) * S, oh * OC:(oh + 1) * OC] = (
            res[core]["out"].astype(np.float32).reshape(S, OC, H, W)
        )
    return out


# revision 23
# speedup vs baseline: 1.6496x; 1.0021x over previous
"""Trainium2 Bass kernel for nn_DiverseRegDCConv2d.

Per-sample dynamic 3x3 conv: filters are generated per sample from an
8-column weight bank (wgen[b] = se[b] @ bank.T), then applied as a
standard 256->256 conv on 28x28 with padding 1.

Sharding (8 cores): 4 batch-groups x 2 out-channel halves. Each core
handles 8 samples x 128 out channels; the weight bank half it needs is
replicated across the 4 batch-groups. No cross-device communication.

Compute strategy (fp8 DoubleRow, 3-pass residual):
  The conv contraction (256 input channels = 2x128) runs as fp8
  DoubleRow matmuls, which process both 128-channel tiles in a single
  instruction at 2x the fp16 rate (4x overall).  Plain e4m3 is not
  accurate enough (rel err 3.7e-2 vs the 2e-2 gate), so operands are
  split hi/lo and three DoubleRow passes accumulate into one PSUM
  group:
      P1: Whi . xhi     (e4m3 x e4m3)  main term
      P2: Whi . xlo     (e4m3 x e4m3)  x-quantization correction
      P3: Wlo . xhi     (e5m2 x e4m3)  W-quantization correction
  Wlo must be e5m2: the residual magnitudes (~1e-3) underflow e4m3's
  denormal range.  Measured end-to-end rel err: ~1.6e-3.

  x windows stream as FLAT 420-wide slices of a row-contiguous
  [128, cc, 31, 30] image (padded, plus a zero guard row), offset by
  ky*30+kx per kernel position; the wrap-around junk lands in output
  columns 28/29 which the evacuation discards.  Flatness is required
  by the DoubleRow ifmap layout ([K, 2, N] with N flat).

  Filters are generated on-device in fp16 (bank tiles x block-diagonal
  se operand -> PSUM fp32, as in the fp16 kernel), then split into the
  e4m3 hi slab (activation copy) and e5m2 lo slab (tensor_tensor
  subtract), spread across Act/DVE/Pool so the split keeps pace with
  the PE during the generation phase.
"""

import sys

for _p in ("/opt/trn_rl_repo", "/root/.axon_site/_ro/trn_rl_repo"):
    if _p not in sys.path:
        sys.path.append(_p)

import numpy as np
import ml_dtypes

import concourse.bass as bass
import concourse.mybir as mybir
from concourse import bacc
from concourse.bass_utils import run_bass_kernel_spmd
from concourse.tile import TileContext

B, C, O, KS, H, W, NUM = 32, 256, 256, 3, 28, 28, 8
P = 128
NCORES = 8
BG, OHALF = 4, 2          # batch-groups x out-channel halves
S = B // BG               # samples per core = 8
OC = O // OHALF           # out channels per core = 128
CC = C // P               # input-channel chunks = 2
G = 16                    # (k,o)-blocks per wgen matmul (with NUM=8 fills K=128)
NP = KS * KS * OC         # (k, o_local) pairs per c-chunk = 1152
NM = NP // G              # wgen matmuls per c-chunk = 72
WR, WC = H + 3, W + 2     # stored x rows (pad + zero guard row) x cols = 31x30
HH = H // 2               # output rows per PSUM group = 14
NW = HH * WC              # flat moving-stream width = 420
F32 = mybir.dt.float32
F16 = mybir.dt.float16
E4 = mybir.dt.float8e4
E5 = mybir.dt.float8e5
IDENT = mybir.ActivationFunctionType.Identity
SUB = mybir.AluOpType.subtract

# -- schedule tuning knobs ------------------------------------------------
WAVE1 = 5          # conv groups interleaved with the wgen phase (PSUM tags)
ST12 = 1           # P1/P2 emitted this many k-blocks behind wgen
ST3 = 2            # P3 lag (waits on the longer hi->sub chain)

_NC = None


def _build_nc():
    nc = bacc.Bacc()
    x_d = nc.declare_dram_parameter("x", [S, 2, CC, P, WR * WC], E4, isOutput=False)
    wp_d = nc.declare_dram_parameter("wp", [P, CC * NM, P], F16, isOutput=False)
    se_d = nc.declare_dram_parameter("sebd", [P, P], F16, isOutput=False)
    b_d = nc.declare_dram_parameter("bias", [P, 1], F32, isOutput=False)
    out_d = nc.declare_dram_parameter("out", [S, P, H * W], F16, isOutput=True)

    with TileContext(nc) as tc:
        with (
            tc.tile_pool(name="constp", bufs=1) as constp,
            tc.tile_pool(name="wstream", bufs=18) as wstream,
            tc.tile_pool(name="slabp", bufs=1) as slabp,
            tc.tile_pool(name="xpool", bufs=1) as xpool,
            tc.tile_pool(name="outp", bufs=4) as outp,
            tc.tile_pool(name="psp", bufs=1, space="PSUM") as psp,
        ):
            se_sb = constp.tile([P, P], F16)
            nc.sync.dma_start(out=se_sb, in_=se_d[:, :])
            bias_sb = constp.tile([P, 1], F32)  # DMA deferred past startup

            # filter slabs: [c_part, cc, k, s, o]; conv lhsT slices are
            # wg*[:, :, k, s, :] = [128, 2, 128], the DoubleRow pair shape.
            wg_hi = slabp.tile([P, CC, KS * KS, S, P], E4)
            wg_lo = slabp.tile([P, CC, KS * KS, S, P], E5)

            # x tiles: [c_part, hi/lo, cc, 930] flat row-contiguous images
            xt = [
                xpool.tile([P, 2, CC, WR * WC], E4, name=f"xt_{s}", tag=f"xt_{s}")
                for s in range(S)
            ]
            xdone = set()

            def emit_xload(s):
                if s in xdone:
                    return
                xdone.add(s)
                nc.sync.dma_start(
                    out=xt[s], in_=x_d[s].rearrange("hl cc p w -> p hl cc w"),
                )

            def emit_wload(cc, k, split=False):
                t0 = cc * NM + k * 8
                wtb = wstream.tile([P, 8, P], F16, name=f"wtb_{cc}_{k}", tag="wtb")
                if split:
                    # halves land separately so the first wgen group starts
                    # as soon as tiles 0-3 arrive
                    nc.sync.dma_start(out=wtb[:, 0:4, :], in_=wp_d[:, t0:t0 + 4, :])
                    nc.sync.dma_start(out=wtb[:, 4:8, :], in_=wp_d[:, t0 + 4:t0 + 8, :])
                else:
                    nc.sync.dma_start(out=wtb, in_=wp_d[:, t0:t0 + 8, :])
                return wtb

            # wgen psum rotates over 3 of the shared pool's 8 bank tags;
            # phase-2 conv groups inherit those tags once wgen drains
            _splitn = [0]

            def emit_wgen(cc, k, wtb):
                # produce wg_hi/wg_lo[:, cc, k, :, :] (8 o_hi blocks = 2 groups)
                for j in range(2):
                    n = _splitn[0]
                    _splitn[0] += 1
                    ps = psp.tile([P, 4 * P], F32, name=f"wgps_{n}",
                                  tag=f"wg{n % 3}")
                    for i in range(4):
                        nc.tensor.matmul(
                            ps[:, i * P:(i + 1) * P], wtb[:, j * 4 + i, :],
                            se_sb, start=True, stop=True,
                        )
                    # psum free layout: (o_hi, s, g); slab wants (s, o_hi, g)
                    oh0 = j * 4
                    src = ps.rearrange("p (oh s g) -> p oh s g", oh=4, s=S, g=G)
                    dhi = wg_hi[:, cc, k, :, oh0 * G:(oh0 + 4) * G].rearrange(
                        "p s (oh g) -> p oh s g", g=G)
                    dlo = wg_lo[:, cc, k, :, oh0 * G:(oh0 + 4) * G].rearrange(
                        "p s (oh g) -> p oh s g", g=G)
                    nc.scalar.activation(dhi, src, IDENT)
                    nc.vector.tensor_tensor(dlo, src, dhi, SUB)

            # conv psum tag order: 5 dedicated tags during phase 1, then the
            # freed wgen tags join the rotation (8 banks deep in phase 2)

            _cv_tags = [f"cv{i}" for i in range(WAVE1)] + ["wg0", "wg1", "wg2"]
            _tag = [0]

            def conv_psum():
                t = psp.tile([P, NW], F32, name=f"cps_{_tag[0]}",
                              tag=_cv_tags[_tag[0] % len(_cv_tags)])
                _tag[0] += 1
                return t

            def emit_conv_mm(k, s, hi, pst, pas, first, last):
                # pas 0: Whi.xhi  1: Whi.xlo  2: Wlo.xhi
                ky, kx = k // KS, k % KS
                off = (hi * HH + ky) * WC + kx
                hl = 1 if pas == 1 else 0
                wg = wg_lo if pas == 2 else wg_hi
                nc.tensor.matmul(
                    pst, wg[:, :, k, s, :], xt[s][:, hl, :, off:off + NW],
                    start=first, stop=last,
                    perf_mode=mybir.MatmulPerfMode.DoubleRow,
                    skip_group_check=True,
                )

            _evacn = [0]
            _ots = {}

            def emit_group_evac(s, hi, pst):
                if s not in _ots:
                    _ots[s] = outp.tile([P, 2, HH * W], F16, name=f"ot_{s}",
                                        tag=f"ot_{s % 4}")
                ot = _ots[s]
                src = pst.rearrange("p (r c) -> p r c", r=HH)[:, :, 0:W]
                dst = ot[:, hi, :].rearrange("p (r c) -> p r c", r=HH)
                if _evacn[0] % 2 == 0:
                    nc.scalar.activation(dst, src, IDENT, bias=bias_sb[:, 0:1])
                else:
                    nc.vector.tensor_scalar(
                        out=dst, in0=src, scalar1=bias_sb[:, 0:1],
                        scalar2=None, op0=mybir.AluOpType.add)
                _evacn[0] += 1
                if s == S - 1:
                    # last sample: per-half stores so the tail DMA is short
                    nc.sync.dma_start(
                        out=out_d[s, :, hi * HH * W:(hi + 1) * HH * W],
                        in_=ot[:, hi, :])
                elif hi == 1:
                    nc.sync.dma_start(out=out_d[s], in_=ot.rearrange("p h w -> p (h w)"))
                return ot

            # ---- phase 1: wgen + wave-1 conv groups, interleaved ----
            wave1 = [(s, hi) for s in range(WAVE1 // 2) for hi in range(2)][:WAVE1]
            prog = {}
            done_k = {}

            def open_group(g):
                prog[g] = conv_psum()
                done_k[g] = 0

            def conv_step(g, k, passes):
                # emit `passes` for kernel position k on group g
                s, hi = g
                pst = prog[g]
                for pas in passes:
                    idx = done_k[g]
                    emit_conv_mm(k, s, hi, pst, pas,
                                 first=(idx == 0), last=(idx == 27 - 1))
                    done_k[g] += 1

            for g in wave1:
                open_group(g)

            # DMA priority order: first wp pair, then early x, then the
            # rest of the weight stream with remaining x interleaved
            wtbs = {}
            wtbs[(0, 0)] = emit_wload(0, 0, split=True)
            wtbs[(1, 0)] = emit_wload(1, 0)
            emit_xload(0)
            nc.sync.dma_start(out=bias_sb, in_=b_d[:, :])
            emit_xload(1)
            for k in range(1, KS * KS):
                wtbs[(0, k)] = emit_wload(0, k)
                wtbs[(1, k)] = emit_wload(1, k)
                if k == 1:
                    emit_xload(2)
                elif k == 3:
                    emit_xload(3)
                elif k == 5:
                    emit_xload(4)
                elif k == 7:
                    emit_xload(5)
            emit_xload(6)
            emit_xload(7)

            for k in range(KS * KS):
                emit_wgen(0, k, wtbs[(0, k)])
                emit_wgen(1, k, wtbs[(1, k)])
                if k >= ST12:
                    for g in wave1:
                        conv_step(g, k - ST12, (0, 1))
                if k >= ST3:
                    for g in wave1:
                        conv_step(g, k - ST3, (2,))
            # drain wave-1 per group so PSUM tags free progressively
            for g in wave1:
                for kk in range(KS * KS - ST12, KS * KS):
                    conv_step(g, kk, (0, 1))
                for kk in range(KS * KS - ST3, KS * KS):
                    conv_step(g, kk, (2,))
                emit_group_evac(*g, prog[g])

            # ---- phase 2: remaining groups, straight bursts ----
            rest = [(s, hi) for s in range(S) for hi in range(2)
                    if (s, hi) not in set(wave1)]
            for g in rest[:-1]:
                open_group(g)
                for k in range(KS * KS):
                    conv_step(g, k, (0, 1, 2))
                emit_group_evac(*g, prog[g])

            # final group runs as two 7-row halves so the last store chain
            # overlaps the second half's matmuls instead of trailing them
            s, hi = rest[-1]
            ot = _ots[s]
            for h2 in range(2):
                pst = psp.tile([P, 7 * WC], F32, name=f"cps_t{h2}",
                               tag=_cv_tags[(_tag[0] + h2) % len(_cv_tags)])
                for k in range(KS * KS):
                    ky, kx = k // KS, k % KS
                    off = (hi * HH + h2 * 7 + ky) * WC + kx
                    for pas in range(3):
                        hl = 1 if pas == 1 else 0
                        wg = wg_lo if pas == 2 else wg_hi
                        nc.tensor.matmul(
                            pst, wg[:, :, k, s, :],
                            xt[s][:, hl, :, off:off + 7 * WC],
                            start=(k == 0 and pas == 0),
                            stop=(k == KS * KS - 1 and pas == 2),
                            perf_mode=mybir.MatmulPerfMode.DoubleRow,
                            skip_group_check=True,
                        )
                src = pst.rearrange("p (r c) -> p r c", r=7)[:, :, 0:W]
                dst = ot[:, hi, h2 * 7 * W:(h2 + 1) * 7 * W].rearrange(
                    "p (r c) -> p r c", r=7)
                nc.scalar.activation(dst, src, IDENT, bias=bias_sb[:, 0:1])
                nc.sync.dma_start(
                    out=out_d[s, :, (hi * HH + h2 * 7) * W:(hi * HH + (h2 + 1) * 7) * W],
                    in_=ot[:, hi, h2 * 7 * W:(h2 + 1) * 7 * W])

    nc.compile()
    return nc


def _get_nc():
    global _NC
    if _NC is None:
        _NC = _build_nc()
    return _NC


def _prep_core_inputs(inputs, inputs_se, weight, bias, bg, oh):
    # weight rows: r = o*(C*9) + c*9 + (ky*3+kx)  -> [O, C, 3, 3, NUM]
    wr = weight.reshape(O, C, KS, KS, NUM)
    wo = wr[oh * OC:(oh + 1) * OC]            # [128, 256, 3, 3, 8]
    p_arr = np.arange(NP)
    k_arr = p_arr // OC                       # k index per (m,g) pair
    o_arr = p_arr % OC
    t = wo[o_arr, :, k_arr // KS, k_arr % KS, :]     # [1152, 256, 8]
    wp = (
        t.reshape(NM, G, CC, P, NUM)
        .transpose(2, 0, 4, 1, 3)             # cc, m, n, g, c
        .reshape(CC * NM, P, P)
        .transpose(1, 0, 2)                   # p-major for contiguous DMA
    )
    wp = np.ascontiguousarray(wp.astype(np.float16))

    se_core = inputs_se[bg * S:(bg + 1) * S]  # [8, 8] (s, n)
    sebd = np.zeros((NUM, G, S, G), dtype=np.float32)
    for g in range(G):
        sebd[:, g, :, g] = se_core.T
    sebd = sebd.reshape(P, P).astype(np.float16)

    # padded x + zero guard row, flat rows; hi/lo e4m3 split
    xp = np.pad(
        inputs[bg * S:(bg + 1) * S], ((0, 0), (0, 0), (1, 2), (1, 1))
    ).reshape(S, CC, P, WR * WC)
    xhi = xp.astype(ml_dtypes.float8_e4m3)
    xlo = (xp - xhi.astype(np.float32)).astype(ml_dtypes.float8_e4m3)
    x_arr = np.stack([xhi, xlo], axis=1)      # [S, 2, CC, P, 930]

    return {
        "x": np.ascontiguousarray(x_arr),
        "wp": wp,
        "sebd": sebd,
        "bias": np.ascontiguousarray(
            bias[oh * OC:(oh + 1) * OC].reshape(OC, 1), dtype=np.float32
        ),
    }


def kernel(inputs, inputs_se, weight, bias):
    inputs = np.asarray(inputs, dtype=np.float32)
    inputs_se = np.asarray(inputs_se, dtype=np.float32)
    weight = np.asarray(weight, dtype=np.float32)
    bias = np.asarray(bias, dtype=np.float32)

    nc = _get_nc()
    in_maps = []
    for core in range(NCORES):
        bg, oh = core // OHALF, core % OHALF
        in_maps.append(_prep_core_inputs(inputs, inputs_se, weight, bias, bg, oh))

    res = run_bass_kernel_spmd(nc, in_maps, list(range(NCORES))).results

    out = np.empty((B, O, H, W), dtype=np.float32)
    for core in range(NCORES):
        bg, oh = core // OHALF, core % OHALF
        out[bg * S:(bg + 1) * S, oh * OC:(oh + 1) * OC] = (
            res[core]["out"].astype(np.float32).reshape(S, OC, H, W)
        )
    return out


# revision 24
# speedup vs baseline: 1.6653x; 1.0095x over previous
"""Trainium2 Bass kernel for nn_DiverseRegDCConv2d.

Per-sample dynamic 3x3 conv: filters are generated per sample from an
8-column weight bank (wgen[b] = se[b] @ bank.T), then applied as a
standard 256->256 conv on 28x28 with padding 1.

Sharding (8 cores): 4 batch-groups x 2 out-channel halves. Each core
handles 8 samples x 128 out channels; the weight bank half it needs is
replicated across the 4 batch-groups. No cross-device communication.

Compute strategy (fp8 DoubleRow, 3-pass residual):
  The conv contraction (256 input channels = 2x128) runs as fp8
  DoubleRow matmuls, which process both 128-channel tiles in a single
  instruction at 2x the fp16 rate (4x overall).  Plain e4m3 is not
  accurate enough (rel err 3.7e-2 vs the 2e-2 gate), so operands are
  split hi/lo and three DoubleRow passes accumulate into one PSUM
  group:
      P1: Whi . xhi     (e4m3 x e4m3)  main term
      P2: Whi . xlo     (e4m3 x e4m3)  x-quantization correction
      P3: Wlo . xhi     (e5m2 x e4m3)  W-quantization correction
  Wlo must be e5m2: the residual magnitudes (~1e-3) underflow e4m3's
  denormal range.  Measured end-to-end rel err: ~1.6e-3.

  x windows stream as FLAT 420-wide slices of a row-contiguous
  [128, cc, 31, 30] image (padded, plus a zero guard row), offset by
  ky*30+kx per kernel position; the wrap-around junk lands in output
  columns 28/29 which the evacuation discards.  Flatness is required
  by the DoubleRow ifmap layout ([K, 2, N] with N flat).

  Filters are generated on-device in fp16 (bank tiles x block-diagonal
  se operand -> PSUM fp32, as in the fp16 kernel), then split into the
  e4m3 hi slab (activation copy) and e5m2 lo slab (tensor_tensor
  subtract), spread across Act/DVE/Pool so the split keeps pace with
  the PE during the generation phase.
"""

import sys

for _p in ("/opt/trn_rl_repo", "/root/.axon_site/_ro/trn_rl_repo"):
    if _p not in sys.path:
        sys.path.append(_p)

import numpy as np
import ml_dtypes

import concourse.bass as bass
import concourse.mybir as mybir
from concourse import bacc
from concourse.bass_utils import run_bass_kernel_spmd
from concourse.tile import TileContext

B, C, O, KS, H, W, NUM = 32, 256, 256, 3, 28, 28, 8
P = 128
NCORES = 8
BG, OHALF = 4, 2          # batch-groups x out-channel halves
S = B // BG               # samples per core = 8
OC = O // OHALF           # out channels per core = 128
CC = C // P               # input-channel chunks = 2
G = 16                    # (k,o)-blocks per wgen matmul (with NUM=8 fills K=128)
NP = KS * KS * OC         # (k, o_local) pairs per c-chunk = 1152
NM = NP // G              # wgen matmuls per c-chunk = 72
WR, WC = H + 3, W + 2     # stored x rows (pad + zero guard row) x cols = 31x30
HH = H // 2               # output rows per PSUM group = 14
NW = HH * WC              # flat moving-stream width = 420
F32 = mybir.dt.float32
F16 = mybir.dt.float16
E4 = mybir.dt.float8e4
E5 = mybir.dt.float8e5
IDENT = mybir.ActivationFunctionType.Identity
SUB = mybir.AluOpType.subtract

# -- schedule tuning knobs ------------------------------------------------
WAVE1 = 5          # conv groups interleaved with the wgen phase (PSUM tags)
ST12 = 1           # P1/P2 emitted this many k-blocks behind wgen
ST3 = 2            # P3 lag (waits on the longer hi->sub chain)

_NC = None


def _build_nc():
    nc = bacc.Bacc()
    x_d = nc.declare_dram_parameter("x", [S, 2, CC, P, WR * WC], E4, isOutput=False)
    wp_d = nc.declare_dram_parameter("wp", [P, CC * NM, P], F16, isOutput=False)
    se_d = nc.declare_dram_parameter("sebd", [P, P], F16, isOutput=False)
    b_d = nc.declare_dram_parameter("bias", [P, 1], F32, isOutput=False)
    out_d = nc.declare_dram_parameter("out", [S, P, H * W], F16, isOutput=True)

    with TileContext(nc) as tc:
        with (
            tc.tile_pool(name="constp", bufs=1) as constp,
            tc.tile_pool(name="wstream", bufs=18) as wstream,
            tc.tile_pool(name="slabp", bufs=1) as slabp,
            tc.tile_pool(name="xpool", bufs=1) as xpool,
            tc.tile_pool(name="outp", bufs=4) as outp,
            tc.tile_pool(name="psp", bufs=1, space="PSUM") as psp,
        ):
            # se rides the Pool/SWDGE queue so its fixed issue overhead
            # overlaps the HWDGE chain of the first weight-block load
            se_sb = constp.tile([P, P], F16)
            nc.gpsimd.dma_start(out=se_sb, in_=se_d[:, :])
            bias_sb = constp.tile([P, 1], F32)  # DMA deferred past startup

            # filter slabs: [c_part, cc, k, s, o]; conv lhsT slices are
            # wg*[:, :, k, s, :] = [128, 2, 128], the DoubleRow pair shape.
            wg_hi = slabp.tile([P, CC, KS * KS, S, P], E4)
            wg_lo = slabp.tile([P, CC, KS * KS, S, P], E5)

            # x tiles: [c_part, hi/lo, cc, 930] flat row-contiguous images
            xt = [
                xpool.tile([P, 2, CC, WR * WC], E4, name=f"xt_{s}", tag=f"xt_{s}")
                for s in range(S)
            ]
            xdone = set()

            def emit_xload(s):
                if s in xdone:
                    return
                xdone.add(s)
                nc.sync.dma_start(
                    out=xt[s], in_=x_d[s].rearrange("hl cc p w -> p hl cc w"),
                )

            def emit_wload(cc, k, split=False):
                t0 = cc * NM + k * 8
                wtb = wstream.tile([P, 8, P], F16, name=f"wtb_{cc}_{k}", tag="wtb")
                if split:
                    # halves land separately so the first wgen group starts
                    # as soon as tiles 0-3 arrive
                    nc.sync.dma_start(out=wtb[:, 0:4, :], in_=wp_d[:, t0:t0 + 4, :])
                    nc.sync.dma_start(out=wtb[:, 4:8, :], in_=wp_d[:, t0 + 4:t0 + 8, :])
                else:
                    nc.sync.dma_start(out=wtb, in_=wp_d[:, t0:t0 + 8, :])
                return wtb

            # wgen psum rotates over 3 of the shared pool's 8 bank tags;
            # phase-2 conv groups inherit those tags once wgen drains
            _splitn = [0]

            def emit_wgen(cc, k, wtb):
                # produce wg_hi/wg_lo[:, cc, k, :, :] (8 o_hi blocks = 2 groups)
                for j in range(2):
                    n = _splitn[0]
                    _splitn[0] += 1
                    ps = psp.tile([P, 4 * P], F32, name=f"wgps_{n}",
                                  tag=f"wg{n % 3}")
                    for i in range(4):
                        nc.tensor.matmul(
                            ps[:, i * P:(i + 1) * P], wtb[:, j * 4 + i, :],
                            se_sb, start=True, stop=True,
                        )
                    # psum free layout: (o_hi, s, g); slab wants (s, o_hi, g)
                    oh0 = j * 4
                    src = ps.rearrange("p (oh s g) -> p oh s g", oh=4, s=S, g=G)
                    dhi = wg_hi[:, cc, k, :, oh0 * G:(oh0 + 4) * G].rearrange(
                        "p s (oh g) -> p oh s g", g=G)
                    dlo = wg_lo[:, cc, k, :, oh0 * G:(oh0 + 4) * G].rearrange(
                        "p s (oh g) -> p oh s g", g=G)
                    nc.scalar.activation(dhi, src, IDENT)
                    nc.vector.tensor_tensor(dlo, src, dhi, SUB)

            # conv psum tag order: 5 dedicated tags during phase 1, then the
            # freed wgen tags join the rotation (8 banks deep in phase 2)

            _cv_tags = [f"cv{i}" for i in range(WAVE1)] + ["wg0", "wg1", "wg2"]
            _tag = [0]

            def conv_psum():
                t = psp.tile([P, NW], F32, name=f"cps_{_tag[0]}",
                              tag=_cv_tags[_tag[0] % len(_cv_tags)])
                _tag[0] += 1
                return t

            def emit_conv_mm(k, s, hi, pst, pas, first, last):
                # pas 0: Whi.xhi  1: Whi.xlo  2: Wlo.xhi
                ky, kx = k // KS, k % KS
                off = (hi * HH + ky) * WC + kx
                hl = 1 if pas == 1 else 0
                wg = wg_lo if pas == 2 else wg_hi
                nc.tensor.matmul(
                    pst, wg[:, :, k, s, :], xt[s][:, hl, :, off:off + NW],
                    start=first, stop=last,
                    perf_mode=mybir.MatmulPerfMode.DoubleRow,
                    skip_group_check=True,
                )

            _evacn = [0]
            _ots = {}

            def emit_group_evac(s, hi, pst):
                if s not in _ots:
                    _ots[s] = outp.tile([P, 2, HH * W], F16, name=f"ot_{s}",
                                        tag=f"ot_{s % 4}")
                ot = _ots[s]
                src = pst.rearrange("p (r c) -> p r c", r=HH)[:, :, 0:W]
                dst = ot[:, hi, :].rearrange("p (r c) -> p r c", r=HH)
                if _evacn[0] % 2 == 0:
                    nc.scalar.activation(dst, src, IDENT, bias=bias_sb[:, 0:1])
                else:
                    nc.vector.tensor_scalar(
                        out=dst, in0=src, scalar1=bias_sb[:, 0:1],
                        scalar2=None, op0=mybir.AluOpType.add)
                _evacn[0] += 1
                if s == S - 1:
                    # last sample: per-half stores so the tail DMA is short
                    nc.sync.dma_start(
                        out=out_d[s, :, hi * HH * W:(hi + 1) * HH * W],
                        in_=ot[:, hi, :])
                elif hi == 1:
                    nc.sync.dma_start(out=out_d[s], in_=ot.rearrange("p h w -> p (h w)"))
                return ot

            # ---- phase 1: wgen + wave-1 conv groups, interleaved ----
            wave1 = [(s, hi) for s in range(WAVE1 // 2) for hi in range(2)][:WAVE1]
            prog = {}
            done_k = {}

            def open_group(g):
                prog[g] = conv_psum()
                done_k[g] = 0

            def conv_step(g, k, passes):
                # emit `passes` for kernel position k on group g
                s, hi = g
                pst = prog[g]
                for pas in passes:
                    idx = done_k[g]
                    emit_conv_mm(k, s, hi, pst, pas,
                                 first=(idx == 0), last=(idx == 27 - 1))
                    done_k[g] += 1

            for g in wave1:
                open_group(g)

            # DMA priority order: first wp pair, then early x, then the
            # rest of the weight stream with remaining x interleaved
            wtbs = {}
            wtbs[(0, 0)] = emit_wload(0, 0, split=True)
            wtbs[(1, 0)] = emit_wload(1, 0)
            emit_xload(0)
            nc.sync.dma_start(out=bias_sb, in_=b_d[:, :])
            emit_xload(1)
            for k in range(1, KS * KS):
                wtbs[(0, k)] = emit_wload(0, k)
                wtbs[(1, k)] = emit_wload(1, k)
                if k == 1:
                    emit_xload(2)
                elif k == 3:
                    emit_xload(3)
                elif k == 5:
                    emit_xload(4)
                elif k == 7:
                    emit_xload(5)
            emit_xload(6)
            emit_xload(7)

            for k in range(KS * KS):
                emit_wgen(0, k, wtbs[(0, k)])
                emit_wgen(1, k, wtbs[(1, k)])
                if k >= ST12:
                    for g in wave1:
                        conv_step(g, k - ST12, (0, 1))
                if k >= ST3:
                    for g in wave1:
                        conv_step(g, k - ST3, (2,))
            # drain wave-1 per group so PSUM tags free progressively
            for g in wave1:
                for kk in range(KS * KS - ST12, KS * KS):
                    conv_step(g, kk, (0, 1))
                for kk in range(KS * KS - ST3, KS * KS):
                    conv_step(g, kk, (2,))
                emit_group_evac(*g, prog[g])

            # ---- phase 2: remaining groups, straight bursts ----
            rest = [(s, hi) for s in range(S) for hi in range(2)
                    if (s, hi) not in set(wave1)]
            for g in rest[:-1]:
                open_group(g)
                for k in range(KS * KS):
                    conv_step(g, k, (0, 1, 2))
                emit_group_evac(*g, prog[g])

            # final group runs as two 7-row halves so the last store chain
            # overlaps the second half's matmuls instead of trailing them
            s, hi = rest[-1]
            ot = _ots[s]
            for h2 in range(2):
                pst = psp.tile([P, 7 * WC], F32, name=f"cps_t{h2}",
                               tag=_cv_tags[(_tag[0] + h2) % len(_cv_tags)])
                for k in range(KS * KS):
                    ky, kx = k // KS, k % KS
                    off = (hi * HH + h2 * 7 + ky) * WC + kx
                    for pas in range(3):
                        hl = 1 if pas == 1 else 0
                        wg = wg_lo if pas == 2 else wg_hi
                        nc.tensor.matmul(
                            pst, wg[:, :, k, s, :],
                            xt[s][:, hl, :, off:off + 7 * WC],
                            start=(k == 0 and pas == 0),
                            stop=(k == KS * KS - 1 and pas == 2),
                            perf_mode=mybir.MatmulPerfMode.DoubleRow,
                            skip_group_check=True,
                        )
                src = pst.rearrange("p (r c) -> p r c", r=7)[:, :, 0:W]
                dst = ot[:, hi, h2 * 7 * W:(h2 + 1) * 7 * W].rearrange(
                    "p (r c) -> p r c", r=7)
                nc.scalar.activation(dst, src, IDENT, bias=bias_sb[:, 0:1])
                nc.sync.dma_start(
                    out=out_d[s, :, (hi * HH + h2 * 7) * W:(hi * HH + (h2 + 1) * 7) * W],
                    in_=ot[:, hi, h2 * 7 * W:(h2 + 1) * 7 * W])

    nc.compile()
    return nc


def _get_nc():
    global _NC
    if _NC is None:
        _NC = _build_nc()
    return _NC


def _prep_core_inputs(inputs, inputs_se, weight, bias, bg, oh):
    # weight rows: r = o*(C*9) + c*9 + (ky*3+kx)  -> [O, C, 3, 3, NUM]
    wr = weight.reshape(O, C, KS, KS, NUM)
    wo = wr[oh * OC:(oh + 1) * OC]            # [128, 256, 3, 3, 8]
    p_arr = np.arange(NP)
    k_arr = p_arr // OC                       # k index per (m,g) pair
    o_arr = p_arr % OC
    t = wo[o_arr, :, k_arr // KS, k_arr % KS, :]     # [1152, 256, 8]
    wp = (
        t.reshape(NM, G, CC, P, NUM)
        .transpose(2, 0, 4, 1, 3)             # cc, m, n, g, c
        .reshape(CC * NM, P, P)
        .transpose(1, 0, 2)                   # p-major for contiguous DMA
    )
    wp = np.ascontiguousarray(wp.astype(np.float16))

    se_core = inputs_se[bg * S:(bg + 1) * S]  # [8, 8] (s, n)
    sebd = np.zeros((NUM, G, S, G), dtype=np.float32)
    for g in range(G):
        sebd[:, g, :, g] = se_core.T
    sebd = sebd.reshape(P, P).astype(np.float16)

    # padded x + zero guard row, flat rows; hi/lo e4m3 split
    xp = np.pad(
        inputs[bg * S:(bg + 1) * S], ((0, 0), (0, 0), (1, 2), (1, 1))
    ).reshape(S, CC, P, WR * WC)
    xhi = xp.astype(ml_dtypes.float8_e4m3)
    xlo = (xp - xhi.astype(np.float32)).astype(ml_dtypes.float8_e4m3)
    x_arr = np.stack([xhi, xlo], axis=1)      # [S, 2, CC, P, 930]

    return {
        "x": np.ascontiguousarray(x_arr),
        "wp": wp,
        "sebd": sebd,
        "bias": np.ascontiguousarray(
            bias[oh * OC:(oh + 1) * OC].reshape(OC, 1), dtype=np.float32
        ),
    }


def kernel(inputs, inputs_se, weight, bias):
    inputs = np.asarray(inputs, dtype=np.float32)
    inputs_se = np.asarray(inputs_se, dtype=np.float32)
    weight = np.asarray(weight, dtype=np.float32)
    bias = np.asarray(bias, dtype=np.float32)

    nc = _get_nc()
    in_maps = []
    for core in range(NCORES):
        bg, oh = core // OHALF, core % OHALF
        in_maps.append(_prep_core_inputs(inputs, inputs_se, weight, bias, bg, oh))

    res = run_bass_kernel_spmd(nc, in_maps, list(range(NCORES))).results

    out = np.empty((B, O, H, W), dtype=np.float32)
    for core in range(NCORES):
        bg, oh = core // OHALF, core % OHALF
        out[bg * S:(bg + 1) * S, oh * OC:(oh + 1) * OC] = (
            res[core]["out"].astype(np.float32).reshape(S, OC, H, W)
        )
    return out
